# revision 1
# baseline (speedup 1.0000x reference)
"""CrystalGCN (3x CGConv + mean-pool + linear) Trainium2 Bass kernel, 8-core SPMD.

Wall-clock-optimized end-to-end path:
  - Host: vectorized serpentine packing (graphs->cores by edge count,
    nodes->32 windows/core by in-degree), edge permutation into per-window
    128-edge tiles. edge_attr ships as fp8-e4m3 (halves the dominant
    upload); indices ship de-replicated ([16, n/16] int16) and are
    replicated across SBUF partitions on device; all replicated weights +
    emb + iota/ident pack into one [1128, 256] fp16 block sharded across
    cores and AllGathered on device.
  - Device per layer: per-128-edge tile, transpose-gather h[dst], h[src]
    (fp16 lhsT), 3 PSUM-accumulated matmuls (dst, src, fp8 edge_attr+bias)
    -> pre[128e, 256]; sigmoid/softplus via exp/ln; scatter-add via
    selection-matrix matmul into per-window PSUM; window flush
    relu(h+acc). AllGather fp16 h shards between layers.
  - Runner: the jax.jit(shard_map) callable is built once and cached;
    big inputs are device_put ASAP (async under axon) so upload overlaps
    the rest of host preprocessing.
"""
import numpy as np
import ml_dtypes
import jax
from jax.sharding import Mesh, PartitionSpec, NamedSharding
from jax.experimental.shard_map import shard_map

import concourse.bacc as bacc
import concourse.mybir as mybir
import concourse.tile as tile
from concourse import library_config
from concourse.bass2jax import (
    _bass_exec_p,
    install_neuronx_cc_hook,
    partition_id_tensor,
)

FP32 = mybir.dt.float32
FP16 = mybir.dt.float16
FP8 = mybir.dt.float8e4
I16 = mybir.dt.int16
AF = mybir.ActivationFunctionType
OP = mybir.AluOpType

N_CORES = 8
N_NODES = 32000
N_EDGES = 320000
N_GRAPHS = 1600
HID = 128
RBF = 32
NODES_PC = 4096          # node slots per core
WINDOWS_PC = 32          # windows per core (128 nodes each)
GRAPHS_PC = 256          # graph slots per core
V_PAD = N_CORES * NODES_PC

_f16 = ml_dtypes.float16 if hasattr(ml_dtypes, "float16") else np.float16
_f8 = mybir.dt.np(mybir.dt.float8e4)

# packed replicated-weights block: row offsets in a [WR, 256] fp16 table
R_WDST, R_WSRC, R_WEA, R_EW, R_IO = 0, 384, 768, 867, 995
WR = 1128  # 1123 used, padded to a multiple of 8


# ---------------------------------------------------------------- host prep --
def preprocess(x, edge_index, edge_attr, batch, ea8_fut=None, put_cb=None):
    """Vectorized partitioning + per-core device arrays (as [8, ...] stacks).

    ea8_fut: optional Future yielding edge_attr pre-cast to fp8 (the cast is
    input-independent of the partitioning, so the caller overlaps it)."""
    x = np.asarray(x)
    src = np.asarray(edge_index[0], np.int64)
    dst = np.asarray(edge_index[1], np.int64)
    batch = np.asarray(batch, np.int64)

    deg = np.bincount(dst, minlength=N_NODES)                   # in-degree
    g_nodes = np.bincount(batch, minlength=N_GRAPHS)
    g_edges = np.bincount(batch, weights=deg.astype(np.float64),
                          minlength=N_GRAPHS).astype(np.int64)

    # graphs -> cores: serpentine over edge count (balances edges/core)
    order = np.argsort(-g_edges, kind="stable")
    rr = np.arange(N_GRAPHS, dtype=np.int64)
    col = rr & 7
    core_rank = np.where((rr >> 3) & 1 == 0, col, 7 - col)
    g_core = np.empty(N_GRAPHS, np.int64)
    g_core[order] = core_rank
    # repair rare node-capacity violations
    for _ in range(64):
        core_nodes = np.bincount(g_core, weights=g_nodes.astype(np.float64),
                                 minlength=N_CORES).astype(np.int64)
        c_over = int(np.argmax(core_nodes))
        if core_nodes[c_over] <= NODES_PC:
            break
        c_under = int(np.argmin(core_nodes))
        excess = core_nodes[c_over] - NODES_PC
        ids = np.where((g_core == c_over) & (g_nodes > 0))[0]
        g_move = ids[np.argmin(np.abs(g_nodes[ids] - excess))]
        g_core[g_move] = c_under

    # local graph slot
    o2 = np.argsort(g_core, kind="stable")
    gcounts = np.bincount(g_core, minlength=N_CORES)
    gstarts = np.concatenate(([0], np.cumsum(gcounts)[:-1]))
    g_slot = np.empty(N_GRAPHS, np.int64)
    g_slot[o2] = np.arange(N_GRAPHS) - np.repeat(gstarts, gcounts)

    # nodes -> windows within core: serpentine over in-degree
    node_core = g_core[batch]
    o3 = np.lexsort((-deg, node_core))
    ncounts = np.bincount(node_core, minlength=N_CORES)
    nstarts = np.concatenate(([0], np.cumsum(ncounts)[:-1]))
    r3 = np.arange(N_NODES) - np.repeat(nstarts, ncounts)
    wcol = r3 & 31
    win = np.where((r3 >> 5) & 1 == 0, wcol, 31 - wcol)
    new_id = np.empty(N_NODES, np.int64)
    new_id[o3] = node_core[o3] * NODES_PC + win * 128 + (r3 >> 5)

    # edges keyed by destination window (uint8 keys -> radix sort)
    nd = new_id[dst]
    ns = new_id[src]
    wkey = nd >> 7
    order_e = np.argsort(wkey.astype(np.uint8), kind="stable")
    wcnt = np.bincount(wkey, minlength=N_CORES * WINDOWS_PC)
    t_w = int(np.ceil(wcnt.max() / 128.0))
    t_w += t_w % 2
    epw = t_w * 128
    e_pad = WINDOWS_PC * epw
    tot = N_CORES * e_pad

    starts = np.zeros(N_CORES * WINDOWS_PC + 1, dtype=np.int64)
    np.cumsum(wcnt, out=starts[1:])
    wk_s = wkey[order_e]
    pos = wk_s * epw + (np.arange(N_EDGES) - starts[wk_s])

    # fp8 edge features (+ ones row for the bias matmul), per-window layout
    if ea8_fut is not None:
        ea8 = ea8_fut.result()
    else:
        ea8 = np.asarray(edge_attr, np.float32).astype(_f8)
    ea_rows = np.empty((tot, 33), _f8)
    ea_rows[:, :32] = _f8(0.0)
    ea_rows[:, 32] = _f8(1.0)
    ea_rows[pos, :32] = ea8[order_e]
    eaT = np.ascontiguousarray(
        ea_rows.reshape(N_CORES, e_pad, 33).transpose(0, 2, 1))  # [8,33,e_pad]
    if put_cb is not None:
        eaT = put_cb(eaT)  # launch the dominant upload before building the rest

    srcw = np.zeros(tot, np.int16)
    dstw = np.zeros(tot, np.int16)
    srcw[pos] = ns[order_e].astype(np.int16)
    dstw[pos] = nd[order_e].astype(np.int16)
    ldw = np.full(tot, 255.0, _f16)
    ldw[pos] = (nd[order_e] & 127).astype(_f16)

    src_idx = np.ascontiguousarray(
        srcw.reshape(N_CORES, e_pad // 16, 16).transpose(0, 2, 1))  # [8,16,n/16]
    dst_idx = np.ascontiguousarray(
        dstw.reshape(N_CORES, e_pad // 16, 16).transpose(0, 2, 1))
    ld = np.ascontiguousarray(
        ldw.reshape(N_CORES, e_pad // 128, 128).transpose(0, 2, 1))  # [8,128,t]

    # per-node pooling metadata + embedding index (by new node id)
    inv_cnt = np.zeros(V_PAD, np.float32)
    lg = np.full(V_PAD, 512.0, np.float32)
    cnt = np.maximum(g_nodes, 1).astype(np.float32)
    inv_cnt[new_id] = 1.0 / cnt[batch]
    lg[new_id] = g_slot[batch]
    embi = np.zeros(V_PAD, np.int16)
    embi[new_id] = np.asarray(x, np.int64).astype(np.int16)

    return dict(
        t_w=t_w, e_pad=e_pad, g_core=g_core, g_slot=g_slot,
        eaT=eaT, src_idx=src_idx, dst_idx=dst_idx, ld=ld,
        emb_own_idx=np.ascontiguousarray(
            embi.reshape(N_CORES, NODES_PC // 16, 16).transpose(0, 2, 1)),
        inv_cnt=np.ascontiguousarray(
            inv_cnt.reshape(N_CORES, WINDOWS_PC, 128).transpose(0, 2, 1)),
        lg0=np.ascontiguousarray(
            lg.reshape(N_CORES, WINDOWS_PC, 128).transpose(0, 2, 1).astype(_f16)),
        lg1=np.ascontiguousarray(
            (lg.reshape(N_CORES, WINDOWS_PC, 128).transpose(0, 2, 1) - 128.0)
            .astype(_f16)),
    )


def pack_weights(emb, Wf, bf, Ws, bs, Wlin):
    """[WR, 256] fp16 block: wdst | wsrc | wea(+bias) | emb|wlin | iota|ident."""
    blk = np.zeros((WR, 256), _f16)
    for l in range(3):
        blk[R_WDST + l * 128:R_WDST + (l + 1) * 128] = \
            np.concatenate([Wf[l][0:128], Ws[l][0:128]], 1).astype(_f16)
        blk[R_WSRC + l * 128:R_WSRC + (l + 1) * 128] = \
            np.concatenate([Wf[l][128:256], Ws[l][128:256]], 1).astype(_f16)
        blk[R_WEA + l * 33:R_WEA + l * 33 + 32] = \
            np.concatenate([Wf[l][256:288], Ws[l][256:288]], 1).astype(_f16)
        blk[R_WEA + l * 33 + 32] = \
            np.concatenate([bf[l], bs[l]]).astype(_f16)
    blk[R_EW:R_EW + emb.shape[0], 0:128] = emb.astype(_f16)
    blk[R_EW:R_EW + 128, 128:256] = Wlin.astype(_f16)
    blk[R_IO:R_IO + 128, 0:128] = np.arange(128, dtype=np.float32)[None, :]\
        .repeat(128, 0).astype(_f16)
    blk[R_IO:R_IO + 128, 128:256] = np.eye(128, dtype=np.float32).astype(_f16)
    return blk


# ---------------------------------------------------------------- device ----
def build_program(t_w: int, e_pad: int):
    nc = bacc.Bacc("TRN2", target_bir_lowering=False, debug=False,
                   enable_asserts=False, num_devices=N_CORES)
    n_tiles = e_pad // 128
    GW = 4                       # windows per gather group
    GN = GW * t_w * 128          # idxs per gather
    n_grp = WINDOWS_PC // GW
    grp_all = [list(range(N_CORES))]

    def din(name, shape, dt):
        return nc.dram_tensor(name, shape, dt, kind="ExternalInput").ap()

    wblk_in = din("wblk", [WR // N_CORES, 256], FP16)
    src_idx = din("src_idx", [16, e_pad // 16], I16)
    dst_idx = din("dst_idx", [16, e_pad // 16], I16)
    ld_d = din("ld", [128, n_tiles], FP16)
    eaT_d = din("eaT", [33, e_pad], FP8)
    embo_d = din("emb_own_idx", [16, NODES_PC // 16], I16)
    invc_d = din("inv_cnt", [128, WINDOWS_PC], FP32)
    lg0_d = din("lg0", [128, WINDOWS_PC], FP16)
    lg1_d = din("lg1", [128, WINDOWS_PC], FP16)
    blin_d = din("blin", [128, 1], FP32)
    out_ext = nc.dram_tensor("outT", [128, GRAPHS_PC], FP16,
                             kind="ExternalOutput").ap()

    with tile.TileContext(nc) as tc:
        with (
            tc.tile_pool(name="const", bufs=1) as cpool,
            tc.tile_pool(name="persist", bufs=1) as ppool,
            tc.tile_pool(name="gath", bufs=2) as gpool,
            tc.tile_pool(name="work", bufs=3) as wpool,
            tc.tile_pool(name="dram", bufs=1, space="DRAM") as dr,
        ):
            nc.gpsimd.load_library(library_config.mlp)

            # ---- gather replicated weight block from all cores
            # (collectives cannot read IO tensors; stage via local DRAM)
            wblk_loc = dr.tile([WR // N_CORES, 256], FP16, tag="wblk_loc")
            nc.sync.dma_start(out=wblk_loc[:], in_=wblk_in)
            wblk = dr.tile([WR, 256], FP16, tag="wblk", name="wblk",
                           addr_space="Shared")
            nc.gpsimd.collective_compute(
                "AllGather", OP.bypass, replica_groups=grp_all,
                ins=[wblk_loc[:]], outs=[wblk[:]])

            # ---- SBUF constants
            iota_sb = cpool.tile([128, 128], FP16)
            nc.sync.dma_start(out=iota_sb[:], in_=wblk[R_IO:R_IO + 128, 0:128])
            ident_sb = cpool.tile([128, 128], FP16)
            nc.sync.dma_start(out=ident_sb[:],
                              in_=wblk[R_IO:R_IO + 128, 128:256])
            wdst_sb = cpool.tile([128, 3 * 256], FP16)
            nc.sync.dma_start(out=wdst_sb[:].rearrange("p (l n) -> p l n", l=3),
                              in_=wblk[R_WDST:R_WDST + 384, :]
                              .rearrange("(l p) n -> p l n", p=128))
            wsrc_sb = cpool.tile([128, 3 * 256], FP16)
            nc.sync.dma_start(out=wsrc_sb[:].rearrange("p (l n) -> p l n", l=3),
                              in_=wblk[R_WSRC:R_WSRC + 384, :]
                              .rearrange("(l p) n -> p l n", p=128))
            wea_sb = cpool.tile([33, 3 * 256], FP16)
            for l in range(3):
                nc.sync.dma_start(
                    out=wea_sb[:, l * 256:(l + 1) * 256],
                    in_=wblk[R_WEA + l * 33:R_WEA + (l + 1) * 33, :])
            wlin_sb = cpool.tile([128, 128], FP16)
            nc.sync.dma_start(out=wlin_sb[:],
                              in_=wblk[R_EW:R_EW + 128, 128:256])
            blin_sb = cpool.tile([128, 1], FP32)
            nc.sync.dma_start(out=blin_sb[:], in_=blin_d)
            ld_sb = cpool.tile([128, n_tiles], FP16)
            nc.sync.dma_start(out=ld_sb[:], in_=ld_d)
            invc_sb = cpool.tile([128, WINDOWS_PC], FP32)
            nc.sync.dma_start(out=invc_sb[:], in_=invc_d)
            lg0_sb = cpool.tile([128, WINDOWS_PC], FP16)
            nc.sync.dma_start(out=lg0_sb[:], in_=lg0_d)
            lg1_sb = cpool.tile([128, WINDOWS_PC], FP16)
            nc.sync.dma_start(out=lg1_sb[:], in_=lg1_d)

            # ---- indices: replicate [16, n] across the 8 gpsimd cores
            srci_sb = cpool.tile([128, e_pad // 16], I16)
            dsti_sb = cpool.tile([128, e_pad // 16], I16)
            embo_sb = cpool.tile([128, NODES_PC // 16], I16)
            for k in range(8):
                sl = slice(16 * k, 16 * (k + 1))
                nc.sync.dma_start(out=srci_sb[sl, :], in_=src_idx)
                nc.sync.dma_start(out=dsti_sb[sl, :], in_=dst_idx)
                nc.sync.dma_start(out=embo_sb[sl, :], in_=embo_d)

            # h tables in DRAM (fp16), one per layer input
            tabs = [dr.tile([V_PAD, HID], FP16, tag=f"tab{i}",
                            name=f"tab{i}", addr_space="Shared")
                    for i in range(3)]
            ag_in = dr.tile([NODES_PC, HID], FP16, tag="ag_in")

            # persistent fp32 own-h  [p, w, f]
            h_own = ppool.tile([128, WINDOWS_PC, HID], FP32)
            hn16 = ppool.tile([128, WINDOWS_PC, HID], FP16)

            # ---- phase 0: own h0 (fp16 gather from emb rows of wblk)
            st16 = wpool.tile([128, WINDOWS_PC, HID], FP16, tag="h0st")
            nc.gpsimd.dma_gather(
                st16[:], wblk[R_EW:R_EW + 128, 0:128], embo_sb[:],
                NODES_PC, NODES_PC, elem_size=HID, elem_step=256,
                transpose=False, single_packet=False)
            nc.vector.tensor_copy(h_own[:], st16[:])
            nc.sync.dma_start(
                out=ag_in[:].rearrange("(w p) f -> p w f", p=128),
                in_=st16[:])
            nc.gpsimd.collective_compute(
                "AllGather", OP.bypass, replica_groups=grp_all,
                ins=[ag_in[:]], outs=[tabs[0][:]])

            # ---- layers
            with tc.tile_pool(name="psum_e", bufs=1, space="PSUM") as pse:
                for l in range(3):
                    tab = tabs[l]
                    for grp in range(n_grp):
                        c0 = grp * GN
                        hdT = gpool.tile([128, 1, GN], FP16, tag="hdT")
                        nc.gpsimd.dma_gather(
                            hdT[:], tab[:], dsti_sb[:, c0 // 16:(c0 + GN) // 16],
                            GN, GN, elem_size=HID, transpose=True,
                            single_packet=False)
                        hsT = gpool.tile([128, 1, GN], FP16, tag="hsT")
                        nc.gpsimd.dma_gather(
                            hsT[:], tab[:], srci_sb[:, c0 // 16:(c0 + GN) // 16],
                            GN, GN, elem_size=HID, transpose=True,
                            single_packet=False)
                        eag = gpool.tile([33, GN], FP8, tag="eag")
                        nc.sync.dma_start(out=eag[:], in_=eaT_d[:, c0:c0 + GN])

                        for wi in range(GW):
                            w = grp * GW + wi
                            acc = pse.tile([128, HID], FP32, tag="acc", bufs=2)
                            for pr in range(t_w // 2):
                                pre = pse.tile([128, 512], FP32, tag="pre",
                                               bufs=3)
                                S = wpool.tile([128, 256], FP16, tag="S")
                                for hf in range(2):
                                    ti = pr * 2 + hf
                                    e0 = wi * t_w * 128 + ti * 128
                                    te = w * t_w + ti
                                    po = pre[:, hf * 256:(hf + 1) * 256]
                                    nc.tensor.matmul(
                                        po, lhsT=hdT[:, 0, e0:e0 + 128],
                                        rhs=wdst_sb[:, l * 256:(l + 1) * 256],
                                        start=True, stop=False)
                                    nc.tensor.matmul(
                                        po, lhsT=hsT[:, 0, e0:e0 + 128],
                                        rhs=wsrc_sb[:, l * 256:(l + 1) * 256],
                                        start=False, stop=False)
                                    nc.tensor.matmul(
                                        po, lhsT=eag[:, e0:e0 + 128],
                                        rhs=wea_sb[:, l * 256:(l + 1) * 256],
                                        start=False, stop=True)
                                    nc.vector.tensor_tensor(
                                        out=S[:, hf * 128:(hf + 1) * 128],
                                        in0=ld_sb[:, te:te + 1]
                                            .to_broadcast([128, 128]),
                                        in1=iota_sb[:], op=OP.is_equal)
                                pre3 = pre[:].rearrange("p (t h) -> p t h",
                                                        h=256)
                                fb = wpool.tile([128, 256], FP32, tag="fb")
                                nc.scalar.activation(fb[:], pre3[:, :, 0:128],
                                                     AF.Exp, scale=-1.0)
                                sb2 = wpool.tile([128, 256], FP32, tag="sb2")
                                nc.scalar.activation(sb2[:], pre3[:, :, 128:256],
                                                     AF.Exp)
                                nc.vector.tensor_scalar_add(fb[:], fb[:], 1.0)
                                nc.vector.tensor_scalar_add(sb2[:], sb2[:], 1.0)
                                nc.vector.reciprocal(fb[:], fb[:])
                                nc.scalar.activation(sb2[:], sb2[:], AF.Ln)
                                msg = wpool.tile([128, 256], FP16, tag="msg")
                                nc.vector.tensor_mul(msg[:], fb[:], sb2[:])
                                for hf in range(2):
                                    nc.tensor.matmul(
                                        acc[:],
                                        lhsT=S[:, hf * 128:(hf + 1) * 128],
                                        rhs=msg[:, hf * 128:(hf + 1) * 128],
                                        start=(pr == 0 and hf == 0),
                                        stop=(pr == t_w // 2 - 1 and hf == 1))
                            # window flush: h = relu(h + acc)
                            hn = wpool.tile([128, HID], FP32, tag="hn")
                            nc.vector.tensor_add(hn[:], acc[:], h_own[:, w, :])
                            nc.vector.tensor_scalar_max(hn[:], hn[:], 0.0)
                            nc.vector.tensor_copy(h_own[:, w, :], hn[:])
                            if l < 2:
                                nc.vector.tensor_copy(hn16[:, w, :], hn[:])
                    if l < 2:
                        nc.sync.dma_start(
                            out=ag_in[:].rearrange("(w p) f -> p w f", p=128),
                            in_=hn16[:])
                        nc.gpsimd.collective_compute(
                            "AllGather", OP.bypass, replica_groups=grp_all,
                            ins=[ag_in[:]], outs=[tabs[l + 1][:]])

            # ---- pooling + final linear
            with tc.tile_pool(name="psum_p", bufs=1, space="PSUM") as psp:
                pa0 = psp.tile([128, HID], FP32, tag="pa0")
                pa1 = psp.tile([128, HID], FP32, tag="pa1")
                for t in range(WINDOWS_PC):
                    sc = wpool.tile([128, HID], FP16, tag="sc")
                    nc.vector.tensor_mul(
                        sc[:], h_own[:, t, :],
                        invc_sb[:, t:t + 1].to_broadcast([128, HID]))
                    sg = wpool.tile([128, 256], FP16, tag="sg")
                    nc.vector.tensor_tensor(
                        out=sg[:, 0:128],
                        in0=lg0_sb[:, t:t + 1].to_broadcast([128, 128]),
                        in1=iota_sb[:], op=OP.is_equal)
                    nc.vector.tensor_tensor(
                        out=sg[:, 128:256],
                        in0=lg1_sb[:, t:t + 1].to_broadcast([128, 128]),
                        in1=iota_sb[:], op=OP.is_equal)
                    nc.tensor.matmul(pa0[:], lhsT=sg[:, 0:128], rhs=sc[:],
                                     start=(t == 0), stop=(t == WINDOWS_PC - 1))
                    nc.tensor.matmul(pa1[:], lhsT=sg[:, 128:256], rhs=sc[:],
                                     start=(t == 0), stop=(t == WINDOWS_PC - 1))
                pooledT = wpool.tile([128, 256], FP16, tag="pooledT")
                for i, pa in enumerate((pa0, pa1)):
                    pc16 = wpool.tile([128, 128], FP16, tag="pc16")
                    nc.vector.tensor_copy(pc16[:], pa[:])
                    pt = psp.tile([128, 128], FP16, tag="pt")
                    nc.tensor.transpose(out=pt[:], in_=pc16[:],
                                        identity=ident_sb[:])
                    nc.vector.tensor_copy(pooledT[:, i * 128:(i + 1) * 128],
                                          pt[:])
                op_ps = psp.tile([128, GRAPHS_PC], FP32, tag="op")
                nc.tensor.matmul(op_ps[:], lhsT=wlin_sb[:], rhs=pooledT[:],
                                 start=True, stop=True)
                outs = wpool.tile([128, GRAPHS_PC], FP16, tag="outs")
                nc.scalar.activation(outs[:], op_ps[:], AF.Identity,
                                     bias=blin_sb[:, 0:1])
                nc.sync.dma_start(out=out_ext, in_=outs[:])
    nc.finalize()
    return nc


# ---------------------------------------------------------------- runner ----
class _Runner:
    """Caches the jitted shard_map callable for one compiled program."""

    def __init__(self, nc):
        install_neuronx_cc_hook()
        self.nc = nc
        pname = nc.partition_id_tensor.name if nc.partition_id_tensor else None
        in_names, out_names, out_avals, zero_shapes = [], [], [], []
        for alloc in nc.m.functions[0].allocations:
            if not isinstance(alloc, mybir.MemoryLocationSet):
                continue
            name = alloc.memorylocations[0].name
            if alloc.kind == "ExternalInput":
                if name != pname:
                    in_names.append(name)
            elif alloc.kind == "ExternalOutput":
                shape = tuple(alloc.tensor_shape)
                dtype = mybir.dt.np(alloc.dtype)
                out_avals.append(jax.core.ShapedArray(shape, dtype))
                out_names.append(name)
                zero_shapes.append((shape, dtype))
        self.in_names = in_names
        self.out_names = out_names
        self.zero_shapes = zero_shapes
        n_params = len(in_names)
        n_outs = len(out_names)
        in_names_full = in_names + out_names
        if pname is not None:
            in_names_full.append(pname)
        donate = tuple(range(n_params, n_params + n_outs))

        def _body(*args):
            operands = list(args)
            if pname is not None:
                operands.append(partition_id_tensor())
            outs = _bass_exec_p.bind(
                *operands, out_avals=tuple(out_avals),
                in_names=tuple(in_names_full), out_names=tuple(out_names),
                lowering_input_output_aliases=(),
                sim_require_finite=True, sim_require_nnan=True, nc=nc)
            return tuple(outs)

        devices = jax.devices()[:N_CORES]
        self.mesh = Mesh(np.asarray(devices), ("core",))
        self.sharding = NamedSharding(self.mesh, PartitionSpec("core"))
        in_specs = (PartitionSpec("core"),) * (n_params + n_outs)
        out_specs = (PartitionSpec("core"),) * n_outs
        self.sharded = jax.jit(
            shard_map(_body, mesh=self.mesh, in_specs=in_specs,
                      out_specs=out_specs, check_rep=False),
            donate_argnums=donate, keep_unused=True)

    def put(self, arr):
        """Async upload of a [8, d0, ...] per-core stack."""
        return jax.device_put(
            arr.reshape(arr.shape[0] * arr.shape[1], *arr.shape[2:]),
            self.sharding)

    def run(self, arrays):
        ins = []
        for name in self.in_names:
            a = arrays[name]
            if isinstance(a, np.ndarray):
                a = a.reshape(a.shape[0] * a.shape[1], *a.shape[2:])
            ins.append(a)
        zeros = [np.zeros((N_CORES * s[0], *s[1:]), dt)
                 for s, dt in self.zero_shapes]
        outs = self.sharded(*ins, *zeros)
        return {name: np.asarray(outs[i])
                for i, name in enumerate(self.out_names)}


_CACHE = {}
_EXEC = None


def _get_runner(t_w, e_pad):
    key = (t_w, e_pad)
    if key not in _CACHE:
        _CACHE[key] = _Runner(build_program(t_w, e_pad))
    return _CACHE[key]


def _executor():
    global _EXEC
    if _EXEC is None:
        from concurrent.futures import ThreadPoolExecutor
        _EXEC = ThreadPoolExecutor(max_workers=2)
    return _EXEC


# ---------------------------------------------------------------- kernel ----
USE_FUT = True
USE_EARLY_PUT = True


def kernel(x, edge_index, edge_attr, batch, emb,
           Wf1, bf1, Ws1, bs1, Wf2, bf2, Ws2, bs2, Wf3, bf3, Ws3, bs3,
           Wlin, blin, _return_extras=False):
    # fp8 cast of edge_attr is independent of the partitioning: overlap it
    ea8_fut = _executor().submit(
        lambda a: np.asarray(a, np.float32).astype(_f8),
        edge_attr) if USE_FUT else None

    def put_cb(eaT):
        e_pad_ = eaT.shape[2]
        return _get_runner(e_pad_ // (WINDOWS_PC * 128), e_pad_).put(eaT)

    prep = preprocess(x, edge_index, edge_attr, batch, ea8_fut,
                      put_cb if USE_EARLY_PUT else None)
    t_w, e_pad = prep["t_w"], prep["e_pad"]
    runner = _get_runner(t_w, e_pad)

    arrays = {}
    # big arrays first: async upload overlaps the remaining host work
    arrays["eaT"] = (prep["eaT"] if USE_EARLY_PUT
                     else runner.put(prep["eaT"]))
    arrays["src_idx"] = runner.put(prep["src_idx"])
    arrays["dst_idx"] = runner.put(prep["dst_idx"])
    arrays["ld"] = runner.put(prep["ld"])

    emb = np.asarray(emb, np.float32)
    Wf = [np.asarray(w, np.float32) for w in (Wf1, Wf2, Wf3)]
    Ws = [np.asarray(w, np.float32) for w in (Ws1, Ws2, Ws3)]
    bf = [np.asarray(b, np.float32) for b in (bf1, bf2, bf3)]
    bs = [np.asarray(b, np.float32) for b in (bs1, bs2, bs3)]
    wblk = pack_weights(emb, Wf, bf, Ws, bs, np.asarray(Wlin, np.float32))
    arrays["wblk"] = wblk.reshape(N_CORES, WR // N_CORES, 256)
    arrays["blin"] = np.broadcast_to(
        np.asarray(blin, np.float32).reshape(1, 128, 1), (N_CORES, 128, 1))
    arrays["emb_own_idx"] = prep["emb_own_idx"]
    arrays["inv_cnt"] = prep["inv_cnt"]
    arrays["lg0"] = prep["lg0"]
    arrays["lg1"] = prep["lg1"]

    res = runner.run(arrays)
    outT = res["outT"].reshape(N_CORES, 128, GRAPHS_PC)
    out = outT[prep["g_core"], :, prep["g_slot"]].astype(np.float32)
    if _return_extras:
        return out, res
    return out

